# revision 10
# baseline (speedup 1.0000x reference)
"""Distributed GCN(4-layer) + LSTM readout kernel for 8 TRN2 NeuronCores.

Self-contained: hardcodes the problem shapes (N=50000, E=800000, D=H=128,
G=500 graphs x L=100 nodes, C=10) and the 8-way sharding.

Strategy
--------
- Nodes are sharded contiguously across 8 cores at graph boundaries
  (sizes 6300 x4 + 6200 x4), so the per-graph LSTM readout is purely local.
- Norm separability: norm = a[src]*a[dst] with a = deg^-1/2.  Table rows
  store u = a[src] * (h @ W); psum accumulates gathered u rows (staircase
  matmuls) + the local self-loop block (identity matmul) + a rank-1 bias
  b[f]*inva[d] so that h' = a[dst] * relu(psum) exactly.  The a[dst] scale
  is folded into the next layer's projection: u' = a^2 * (relu(psum) @ W),
  applied as a per-partition ACT scale on the PSUM->SBUF copy.
- Per GCN layer, each core writes its u slab to DRAM and AllGathers the 8
  slabs into a replicated bf16 table.  Edge aggregation: edges partitioned
  by dst shard, grouped by 128-dst blocks; scatter-add becomes PSUM matmul
  accumulation psum[f,d] += sum_e GX[e,f] * S[e,d], with GX = dma_gather of
  table rows (by src) and S a 0/1 staircase built via is_equal(seg, iota).
- dma_gather indices are int16, so the table is addressed in two halves
  (cores 0-3 / 4-7) and per-block edge lists are split accordingly.
- LSTM: recurrence does 8 matmuls/step (4 Whh@h gate matmuls + 4 Wih@x_t
  accumulations reading strided columns of the layer-4 activations), one
  fused sigmoid over [i,f,o], tanh for g, 3 DVE ops for c, tanh + 1 DVE op
  for h.  Gate biases (zero in this problem) enter as rank-1 matmuls.
"""
import dataclasses
import os
import numpy as np
import ml_dtypes

import concourse.bass as bass
import concourse.mybir as mybir
import concourse.tile as tile
from concourse import bacc
from concourse.bass_utils import run_bass_kernel_spmd

F32 = mybir.dt.float32
BF16 = mybir.dt.bfloat16
I16 = mybir.dt.int16
P = 128

TRACE = False          # set True (e.g. from test.py) to profile
LAST_RESULTS = None    # BassKernelResults of the last run (for profiling)

AF = mybir.ActivationFunctionType
OP = mybir.AluOpType


@dataclasses.dataclass
class Config:
    N: int = 50000
    E: int = 800000
    D: int = 128
    H: int = 128
    L: int = 100
    C: int = 10
    NCORES: int = 8
    GROUP_BLOCKS: int = 4  # dst blocks per gather super-group

    def __post_init__(self):
        assert self.D == 128 and self.H == 128
        self.GROUP_BLOCKS = int(os.environ.get("GNN_GB", self.GROUP_BLOCKS))
        base = (self.N // self.NCORES) // self.L * self.L
        hi = base + self.L
        n_hi = (self.N - base * self.NCORES) // self.L
        self.sizes = [hi] * n_hi + [base] * (self.NCORES - n_hi)
        assert sum(self.sizes) == self.N
        self.offs = np.concatenate([[0], np.cumsum(self.sizes)]).astype(np.int64)
        self.S_PAD = hi
        self.NBLK = -(-self.S_PAD // P)
        self.SLAB = self.NBLK * P
        self.THALF = (self.NCORES // 2) * self.SLAB
        assert self.THALF <= 32768, "int16 gather index overflow"
        self.NG = self.S_PAD // self.L
        self.G = self.N // self.L


PAD_SEG = 255.0
GATE_PERM = [0, 1, 3, 2]  # (i, f, g, o) -> (i, f, o, g)


def preprocess(cfg, x, edge_index, Ws, bs, W_ih, W_hh, b_ih, b_hh,
               lin_W, lin_b):
    N = cfg.N
    src = np.asarray(edge_index[0], np.int64)
    dst = np.asarray(edge_index[1], np.int64)
    deg = (np.bincount(dst, minlength=N) + 1.0).astype(np.float32)
    a = (1.0 / np.sqrt(deg)).astype(np.float32)

    shard_of = np.searchsorted(cfg.offs[1:], np.arange(N), side="right")
    w_in = np.arange(N) - cfg.offs[shard_of]
    # partition-major table layout: node (block b, lane p) -> row p*NBLK + b
    trow = shard_of * cfg.SLAB + (w_in % P) * cfg.NBLK + (w_in // P)

    e_core = shard_of[dst]
    e_half = (trow[src] >= cfg.THALF).astype(np.int64)
    e_tix = (trow[src] - e_half * cfg.THALF).astype(np.int64)
    e_blk = ((dst - cfg.offs[e_core]) // P).astype(np.int64)
    e_seg = ((dst - cfg.offs[e_core]) % P).astype(np.int64)

    # sort by src table row within (core, half, blk) for HBM locality
    order = np.lexsort((e_tix, e_blk, e_half, e_core))
    src_s = src[order]
    core_s, half_s, tix_s, blk_s, seg_s = (
        arr[order] for arr in (e_core, e_half, e_tix, e_blk, e_seg))

    counts = np.zeros((cfg.NCORES, 2, cfg.NBLK), np.int64)
    np.add.at(counts, (core_s, half_s, blk_s), 1)
    chunks = -(-counts.max(axis=0) // P)
    cA, cB = chunks[0], chunks[1]

    groups = []
    for g0 in range(0, cfg.NBLK, cfg.GROUP_BLOCKS):
        groups.append(list(range(g0, min(g0 + cfg.GROUP_BLOCKS, cfg.NBLK))))

    TA = int(cA.sum()) * P
    TB = int(cB.sum()) * P
    run_off = np.zeros((cfg.NCORES, 2, cfg.NBLK), np.int64)
    run_off.reshape(-1)[1:] = np.cumsum(counts.reshape(-1))[:-1]

    # LSTM weights with gates reordered (i, f, o, g)
    Wih = np.asarray(W_ih, np.float32).reshape(4, P, P)[GATE_PERM]
    Whh = np.asarray(W_hh, np.float32).reshape(4, P, P)[GATE_PERM]
    bg4 = (np.asarray(b_ih, np.float32)
           + np.asarray(b_hh, np.float32)).reshape(4, P)[GATE_PERM]
    WihT = np.concatenate([w.T for w in Wih], axis=1)  # [P, 4P]
    WhhT = np.concatenate([w.T for w in Whh], axis=1)

    sched_flags = dict(
        has_b=any(np.any(np.asarray(b)) for b in bs),
        has_bg=bool(np.any(np.asarray(b_ih)) or np.any(np.asarray(b_hh))),
    )
    in_maps = []
    for c in range(cfg.NCORES):
        idx_flat = {0: np.zeros(TA, np.int64), 1: np.zeros(TB, np.int64)}
        seg_flat = {0: np.full(TA, PAD_SEG, np.float32),
                    1: np.full(TB, PAD_SEG, np.float32)}
        for h_i, tot_c in enumerate((cA, cB)):
            pos = 0
            for b in range(cfg.NBLK):
                n = int(counts[c, h_i, b])
                o = int(run_off[c, h_i, b])
                idx_flat[h_i][pos:pos + n] = tix_s[o:o + n]
                seg_flat[h_i][pos:pos + n] = seg_s[o:o + n]
                pos += int(tot_c[b]) * P
        idxA = np.zeros((P, max(TA // 16, 1)), np.int16)
        idxB = np.zeros((P, max(TB // 16, 1)), np.int16)
        if TA:
            idxA[:] = np.tile(idx_flat[0].reshape(-1, 16).T.astype(np.int16),
                              (8, 1))
        if TB:
            idxB[:] = np.tile(idx_flat[1].reshape(-1, 16).T.astype(np.int16),
                              (8, 1))
        segA = np.zeros((P, max(TA // P, 1)), ml_dtypes.bfloat16)
        segB = np.zeros((P, max(TB // P, 1)), ml_dtypes.bfloat16)
        if TA:
            segA[:] = seg_flat[0].reshape(-1, P).T.astype(ml_dtypes.bfloat16)
        if TB:
            segB[:] = seg_flat[1].reshape(-1, P).T.astype(ml_dtypes.bfloat16)

        o, s = int(cfg.offs[c]), int(cfg.sizes[c])
        a_pad = np.ones(cfg.SLAB, np.float32)
        a_pad[:s] = a[o:o + s]
        xTb = np.zeros((P, cfg.SLAB), ml_dtypes.bfloat16)
        xTb[:, :s] = np.asarray(x[o:o + s], np.float32).T.astype(
            ml_dtypes.bfloat16)
        a_b = np.tile(a_pad, (P, 1)).astype(ml_dtypes.bfloat16)
        aT = np.ascontiguousarray(
            a_pad.reshape(cfg.NBLK, P).T)          # [P, NBLK]
        a2T = np.ascontiguousarray((a_pad * a_pad).reshape(cfg.NBLK, P).T)
        invaT = (1.0 / a_pad).reshape(1, cfg.SLAB).astype(ml_dtypes.bfloat16)

        m = {
            "xTb": xTb, "a_b": a_b, "aT": aT, "a2T": a2T,
            "idxA": idxA, "idxB": idxB, "segA": segA, "segB": segB,
            "iota": np.tile(np.arange(P, dtype=np.float32), (P, 1)).astype(
                ml_dtypes.bfloat16),
            "ident": np.eye(P, dtype=ml_dtypes.bfloat16),
            "WihT": WihT.astype(ml_dtypes.bfloat16),
            "WhhT": WhhT.astype(ml_dtypes.bfloat16),
            "linW": np.asarray(lin_W, np.float32).astype(ml_dtypes.bfloat16),
            "linb": np.asarray(lin_b, np.float32).reshape(1, cfg.C),
            "ones1": np.ones((1, P), np.float32),
        }
        if sched_flags["has_b"]:
            m["invaT"] = invaT
        if sched_flags["has_bg"]:
            m["bgrow"] = np.ascontiguousarray(
                bg4.reshape(1, 4 * P)).astype(np.float32)
        for li in range(4):
            m[f"W{li}"] = np.asarray(Ws[li], np.float32).astype(
                ml_dtypes.bfloat16)
            if sched_flags["has_b"]:
                m[f"brow{li}"] = np.asarray(bs[li], np.float32).reshape(
                    1, P).astype(ml_dtypes.bfloat16)
        in_maps.append(m)

    sched = dict(cA=cA.astype(np.int64), cB=cB.astype(np.int64),
                 groups=groups, TA=TA, TB=TB, **sched_flags)
    return in_maps, sched


def build_program(cfg, sched, trn_type="TRN2", debug=False):
    single_packet = os.environ.get("GNN_SP", "0") == "1"
    nc = bacc.Bacc(trn_type, target_bir_lowering=False, debug=debug,
                   num_devices=cfg.NCORES, num_swdge_queues=4)
    cA, cB, groups = sched["cA"], sched["cB"], sched["groups"]
    TA, TB = sched["TA"], sched["TB"]
    SLAB, NBLK, THALF, NG = cfg.SLAB, cfg.NBLK, cfg.THALF, cfg.NG

    def din(name, shape, dt):
        return nc.dram_tensor(name, shape, dt, kind="ExternalInput")

    has_b = sched["has_b"]
    has_bg = sched["has_bg"]
    xTb_d = din("xTb", [P, SLAB], BF16)
    a_d = din("a_b", [P, SLAB], BF16)
    aT_d = din("aT", [P, NBLK], F32)
    a2T_d = din("a2T", [P, NBLK], F32)
    invaT_d = din("invaT", [1, SLAB], BF16) if has_b else None
    idxA_d = din("idxA", [P, max(TA // 16, 1)], I16)
    idxB_d = din("idxB", [P, max(TB // 16, 1)], I16)
    segA_d = din("segA", [P, max(TA // P, 1)], BF16)
    segB_d = din("segB", [P, max(TB // P, 1)], BF16)
    iota_d = din("iota", [P, P], BF16)
    ident_d = din("ident", [P, P], BF16)
    W_d = [din(f"W{li}", [P, P], BF16) for li in range(4)]
    brow_d = ([din(f"brow{li}", [1, P], BF16) for li in range(4)]
              if has_b else None)
    WihT_d = din("WihT", [P, 4 * P], BF16)
    WhhT_d = din("WhhT", [P, 4 * P], BF16)
    bgrow_d = din("bgrow", [1, 4 * P], F32) if has_bg else None
    linW_d = din("linW", [P, cfg.C], BF16)
    linb_d = din("linb", [1, cfg.C], F32)
    ones1_d = din("ones1", [1, P], F32)
    out_d = nc.dram_tensor("out", [NG, cfg.C], F32, kind="ExternalOutput")

    rg = [list(range(cfg.NCORES))]

    with tile.TileContext(nc) as tc:
        with tc.tile_pool(name="dram", bufs=1, space="DRAM") as dpool, \
             tc.tile_pool(name="const", bufs=1) as cpool, \
             tc.tile_pool(name="state", bufs=1) as spool, \
             tc.tile_pool(name="gat", bufs=3) as gpool, \
             tc.tile_pool(name="work", bufs=2) as wpool, \
             tc.tile_pool(name="pagg", bufs=4, space="PSUM") as ppool, \
             tc.tile_pool(name="pu", bufs=2, space="PSUM") as upool, \
             tc.tile_pool(name="plstm", bufs=2, space="PSUM") as lpool:

            def cload(dram, shape, dt, tag):
                t = cpool.tile(shape, dt, tag=tag)
                nc.sync.dma_start(t[:], dram[:])
                return t
            iota_t = cload(iota_d, [P, P], BF16, "c_iota")
            ident_t = cload(ident_d, [P, P], BF16, "c_ident")
            W_t = [cload(W_d[i], [P, P], BF16, f"c_W{i}") for i in range(4)]
            brow_t = ([cload(brow_d[i], [1, P], BF16, f"c_b{i}")
                       for i in range(4)] if has_b else None)
            WihT_t = cload(WihT_d, [P, 4 * P], BF16, "c_wih")
            WhhT_t = cload(WhhT_d, [P, 4 * P], BF16, "c_whh")
            bgrow_t = (cload(bgrow_d, [1, 4 * P], F32, "c_bgrow")
                       if has_bg else None)
            linW_t = cload(linW_d, [P, cfg.C], BF16, "c_linw")
            linb_t = cload(linb_d, [1, cfg.C], F32, "c_linb")
            ones1_t = cload(ones1_d, [1, P], F32, "c_ones1")
            aT_t = cload(aT_d, [P, NBLK], F32, "c_aT")
            a2T_t = cload(a2T_d, [P, NBLK], F32, "c_a2T")
            invaT_t = (cload(invaT_d, [1, SLAB], BF16, "c_invaT")
                       if has_b else None)
            a_t = cload(a_d, [P, SLAB], BF16, "c_a")

            slab_sb = [spool.tile([P, SLAB], BF16, tag=f"slab{i}",
                                  name=f"slab{i}")
                       for i in range(2)]
            gbuf = spool.tile([P, SLAB], BF16, tag="gbuf")
            h4a = slab_sb[0]  # free after layer 3; reused for LSTM input

            # ---- layer-0 projection: slab0 = a * (x @ W1), [node, feat] ----
            for j in range(0, SLAB, 512):
                w = min(512, SLAB - j)
                xc = wpool.tile([P, 512], BF16, tag="xchunk")
                nc.sync.dma_start(xc[:, :w], xTb_d[:, j:j + w])
                for i in range(w // P):
                    k = j // P + i
                    pu = upool.tile([P, P], F32, tag="u", space="PSUM")
                    nc.tensor.matmul(pu[:], lhsT=xc[:, i * P:(i + 1) * P],
                                     rhs=W_t[0][:], start=True, stop=True)
                    nc.scalar.activation(
                        out=slab_sb[0][:, k * P:(k + 1) * P], in_=pu[:],
                        func=AF.Identity, scale=aT_t[:, k:k + 1])

            def push_table(li):
                par = li % 2
                slab_dram = dpool.tile([SLAB, P], BF16, tag="slab_dram",
                                       bufs=2)
                table_dram = dpool.tile([cfg.NCORES * SLAB, P], BF16,
                                        addr_space="Shared", tag="table",
                                        bufs=2)
                nc.sync.dma_start(
                    slab_dram[:].rearrange("(p b) f -> p b f", b=NBLK),
                    slab_sb[par][:].rearrange("p (b f) -> p b f", f=P))
                nc.gpsimd.collective_compute(
                    "AllGather", mybir.AluOpType.bypass,
                    replica_groups=rg,
                    ins=[slab_dram[:]],
                    outs=[table_dram[:]],
                )
                return table_dram

            table_t = push_table(0)

            gq = 0
            for li in range(4):
                cur = slab_sb[li % 2]
                nxt = slab_sb[(li + 1) % 2]
                ao = 0
                bo = 0
                pending_u = []  # blocks whose next-layer projection is due

                def do_u(b):
                    if li < 3:
                        pu = upool.tile([P, P], F32, tag="u", space="PSUM")
                        nc.tensor.matmul(pu[:],
                                         lhsT=gbuf[:, b * P:(b + 1) * P],
                                         rhs=W_t[li + 1][:],
                                         start=True, stop=True)
                        nc.scalar.activation(
                            out=nxt[:, b * P:(b + 1) * P], in_=pu[:],
                            func=AF.Identity, scale=a2T_t[:, b:b + 1])
                    else:
                        nc.vector.tensor_tensor(
                            out=h4a[:, b * P:(b + 1) * P],
                            in0=gbuf[:, b * P:(b + 1) * P],
                            in1=a_t[:, b * P:(b + 1) * P],
                            op=OP.mult)

                for blks in groups:
                    nca = int(cA[blks].sum())
                    ncb = int(cB[blks].sum())
                    gx = {}
                    stg = {}
                    for half, ncnt, idxd, segd, off in (
                            (0, nca, idxA_d, segA_d, ao),
                            (1, ncb, idxB_d, segB_d, bo)):
                        if ncnt == 0:
                            continue
                        it = gpool.tile([P, ncnt * 8], I16, tag=f"idx{half}")
                        nc.sync.dma_start(
                            it[:], idxd[:, off * 8:(off + ncnt) * 8])
                        g = gpool.tile([P, ncnt, P], BF16, tag=f"gx{half}")
                        nc.gpsimd.dma_gather(
                            out_ap=g[:],
                            in_ap=table_t[half * THALF:(half + 1) * THALF, :],
                            idxs_ap=it[:],
                            num_idxs=ncnt * P,
                            num_idxs_reg=ncnt * P,
                            elem_size=P,
                            single_packet=single_packet,
                            queue_num=gq % 4,
                        )
                        gq += 1
                        gx[half] = g
                        st_ = gpool.tile([P, ncnt], BF16, tag=f"seg{half}")
                        nc.sync.dma_start(st_[:], segd[:, off:off + ncnt])
                        sg = gpool.tile([P, ncnt, P], BF16, tag=f"stg{half}")
                        nc.vector.tensor_tensor(
                            out=sg[:],
                            in0=st_[:].rearrange("p (c o) -> p c o", o=1)
                                .to_broadcast((P, ncnt, P)),
                            in1=iota_t[:].rearrange("p (o f) -> p o f", o=1)
                                .to_broadcast((P, ncnt, P)),
                            op=OP.is_equal)
                        stg[half] = sg

                    ca_in_grp = 0
                    cb_in_grp = 0
                    for b in blks:
                        pb = ppool.tile([P, P], F32, tag="agg", space="PSUM")
                        na, nb_ = int(cA[b]), int(cB[b])
                        # self-loop + rank-1 bias b[f]*inva[d]
                        nc.tensor.matmul(pb[:],
                                         lhsT=cur[:, b * P:(b + 1) * P],
                                         rhs=ident_t[:], start=True,
                                         stop=(not has_b and na + nb_ == 0))
                        if has_b:
                            nc.tensor.matmul(
                                pb[:], lhsT=brow_t[li][:],
                                rhs=invaT_t[:, b * P:(b + 1) * P],
                                start=False, stop=(na + nb_ == 0))
                        done = 0
                        for half, cnt, base in ((0, na, ca_in_grp),
                                                (1, nb_, cb_in_grp)):
                            for ci in range(cnt):
                                col = base + ci
                                done += 1
                                nc.tensor.matmul(
                                    pb[:], lhsT=gx[half][:, col, :],
                                    rhs=stg[half][:, col, :],
                                    start=False, stop=(done == na + nb_))
                        ca_in_grp += na
                        cb_in_grp += nb_
                        nc.scalar.activation(
                            out=gbuf[:, b * P:(b + 1) * P], in_=pb[:],
                            func=AF.Relu)
                        pending_u.append(b)
                        if len(pending_u) > 1:
                            do_u(pending_u.pop(0))
                    ao += nca
                    bo += ncb
                for b in pending_u:
                    do_u(b)
                if li < 3:
                    table_t = push_table(li + 1)

            c_t = spool.tile([P, NG], F32, tag="c")
            h_t = spool.tile([P, NG], BF16, tag="h")
            nc.vector.memset(c_t[:], 0.0)
            nc.vector.memset(h_t[:], 0.0)

            for t in range(cfg.L):
                pg = lpool.tile([P, 4 * NG], F32, tag="lstm", space="PSUM")
                xt = h4a[:, t:cfg.S_PAD:cfg.L]
                for q in range(4):
                    nc.tensor.matmul(
                        pg[:, q * NG:(q + 1) * NG],
                        lhsT=WhhT_t[:, q * P:(q + 1) * P],
                        rhs=h_t[:], start=True, stop=False)
                    if has_bg:
                        nc.tensor.matmul(
                            pg[:, q * NG:(q + 1) * NG],
                            lhsT=bgrow_t[:, q * P:(q + 1) * P],
                            rhs=ones1_t[:, :NG],
                            start=False, stop=False)
                    nc.tensor.matmul(
                        pg[:, q * NG:(q + 1) * NG],
                        lhsT=WihT_t[:, q * P:(q + 1) * P],
                        rhs=xt, start=False, stop=True)
                af = wpool.tile([P, 3 * NG], F32, tag="af")
                nc.scalar.activation(out=af[:], in_=pg[:, :3 * NG],
                                     func=AF.Sigmoid)
                gv = wpool.tile([P, NG], F32, tag="gv")
                nc.scalar.activation(out=gv[:], in_=pg[:, 3 * NG:4 * NG],
                                     func=AF.Tanh)
                ig = wpool.tile([P, NG], F32, tag="ig")
                nc.vector.tensor_tensor(out=ig[:], in0=af[:, :NG], in1=gv[:],
                                        op=OP.mult)
                fc = wpool.tile([P, NG], F32, tag="fc")
                nc.vector.tensor_tensor(out=fc[:], in0=af[:, NG:2 * NG],
                                        in1=c_t[:], op=OP.mult)
                nc.vector.tensor_tensor(out=c_t[:], in0=fc[:], in1=ig[:],
                                        op=OP.add)
                tc_ = wpool.tile([P, NG], F32, tag="tc")
                nc.scalar.activation(out=tc_[:], in_=c_t[:], func=AF.Tanh)
                nc.vector.tensor_tensor(out=h_t[:], in0=af[:, 2 * NG:3 * NG],
                                        in1=tc_[:], op=OP.mult)

            po = lpool.tile([P, cfg.C], F32, tag="lstm", space="PSUM")
            nc.tensor.matmul(po[:NG, :], lhsT=h_t[:, :NG], rhs=linW_t[:],
                             start=True, stop=False)
            nc.tensor.matmul(po[:NG, :], lhsT=ones1_t[:, :NG], rhs=linb_t[:],
                             start=False, stop=True)
            os_ = wpool.tile([P, cfg.C], F32, tag="outs")
            nc.scalar.activation(out=os_[:NG, :], in_=po[:NG, :], func=AF.Copy)
            nc.sync.dma_start(out_d[:], os_[:NG, :])

    nc.compile()
    return nc


def assemble(cfg, results):
    out = np.zeros((cfg.G, cfg.C), np.float32)
    for c in range(cfg.NCORES):
        g0 = int(cfg.offs[c]) // cfg.L
        ng = cfg.sizes[c] // cfg.L
        out[g0:g0 + ng] = results[c]["out"][:ng]
    return out


_BUILD_CACHE = {}


def kernel(x, edge_index, batch, W1, b1, W2, b2, W3, b3, W4, b4,
           W_ih, W_hh, b_ih, b_hh, lin_W, lin_b):
    global LAST_RESULTS
    cfg = Config()
    x = np.asarray(x, np.float32)
    edge_index = np.asarray(edge_index, np.int64)
    Ws = [np.asarray(w, np.float32) for w in (W1, W2, W3, W4)]
    bs = [np.asarray(b, np.float32) for b in (b1, b2, b3, b4)]

    in_maps, sched = preprocess(
        cfg, x, edge_index, Ws, bs,
        np.asarray(W_ih, np.float32), np.asarray(W_hh, np.float32),
        np.asarray(b_ih, np.float32), np.asarray(b_hh, np.float32),
        np.asarray(lin_W, np.float32), np.asarray(lin_b, np.float32))

    key = (sched["TA"], sched["TB"], tuple(sched["cA"]), tuple(sched["cB"]),
           sched["has_b"], sched["has_bg"])
    if key not in _BUILD_CACHE:
        _BUILD_CACHE[key] = build_program(cfg, sched)
    nc = _BUILD_CACHE[key]

    res = run_bass_kernel_spmd(nc, in_maps, core_ids=list(range(cfg.NCORES)),
                               trace=TRACE)
    LAST_RESULTS = res
    return assemble(cfg, res.results)


# revision 12
# speedup vs baseline: 1.1536x; 1.1536x over previous
"""Distributed GCN(4-layer) + LSTM readout kernel for 8 TRN2 NeuronCores.

Self-contained: hardcodes the problem shapes (N=50000, E=800000, D=H=128,
G=500 graphs x L=100 nodes, C=10) and the 8-way sharding.

Strategy
--------
- Nodes are sharded contiguously across 8 cores at graph boundaries
  (sizes 6300 x4 + 6200 x4), so the per-graph LSTM readout is purely local.
- Norm separability: norm = a[src]*a[dst] with a = deg^-1/2.  Table rows
  store u = a[src] * (h @ W); psum accumulates gathered u rows (staircase
  matmuls) + the local self-loop block (identity matmul) + a rank-1 bias
  b[f]*inva[d] so that h' = a[dst] * relu(psum) exactly.  The a[dst] scale
  is folded into the next layer's projection: u' = a^2 * (relu(psum) @ W),
  applied as a per-partition ACT scale on the PSUM->SBUF copy.
- Per GCN layer, each core writes its u slab to DRAM and AllGathers the 8
  slabs into a replicated bf16 table.  Edge aggregation: edges partitioned
  by dst shard, grouped by 128-dst blocks; scatter-add becomes PSUM matmul
  accumulation psum[f,d] += sum_e GX[e,f] * S[e,d], with GX = dma_gather of
  table rows (by src) and S a 0/1 staircase built via is_equal(seg, iota).
- dma_gather indices are int16, so the table is addressed in two halves
  (cores 0-3 / 4-7) and per-block edge lists are split accordingly.
- LSTM: recurrence does 8 matmuls/step (4 Whh@h gate matmuls + 4 Wih@x_t
  accumulations reading strided columns of the layer-4 activations), one
  fused sigmoid over [i,f,o], tanh for g, 3 DVE ops for c, tanh + 1 DVE op
  for h.  Gate biases (zero in this problem) enter as rank-1 matmuls.
"""
import dataclasses
import os
import numpy as np
import ml_dtypes

import concourse.bass as bass
import concourse.mybir as mybir
import concourse.tile as tile
from concourse import bacc
from concourse.bass_utils import run_bass_kernel_spmd

F32 = mybir.dt.float32
BF16 = mybir.dt.bfloat16
I16 = mybir.dt.int16
P = 128

TRACE = False          # set True (e.g. from test.py) to profile
LAST_RESULTS = None    # BassKernelResults of the last run (for profiling)

AF = mybir.ActivationFunctionType
OP = mybir.AluOpType


@dataclasses.dataclass
class Config:
    N: int = 50000
    E: int = 800000
    D: int = 128
    H: int = 128
    L: int = 100
    C: int = 10
    NCORES: int = 8
    GROUP_BLOCKS: int = 4  # dst blocks per gather super-group

    def __post_init__(self):
        assert self.D == 128 and self.H == 128
        self.GROUP_BLOCKS = int(os.environ.get("GNN_GB", self.GROUP_BLOCKS))
        base = (self.N // self.NCORES) // self.L * self.L
        hi = base + self.L
        n_hi = (self.N - base * self.NCORES) // self.L
        self.sizes = [hi] * n_hi + [base] * (self.NCORES - n_hi)
        assert sum(self.sizes) == self.N
        self.offs = np.concatenate([[0], np.cumsum(self.sizes)]).astype(np.int64)
        self.S_PAD = hi
        self.NBLK = -(-self.S_PAD // P)
        self.SLAB = self.NBLK * P
        self.THALF = (self.NCORES // 2) * self.SLAB
        assert self.THALF <= 32768, "int16 gather index overflow"
        self.NG = self.S_PAD // self.L
        self.G = self.N // self.L


PAD_SEG = 255.0
GATE_PERM = [0, 1, 3, 2]  # (i, f, g, o) -> (i, f, o, g)


def preprocess(cfg, x, edge_index, Ws, bs, W_ih, W_hh, b_ih, b_hh,
               lin_W, lin_b):
    N = cfg.N
    src = np.asarray(edge_index[0], np.int64)
    dst = np.asarray(edge_index[1], np.int64)
    deg = (np.bincount(dst, minlength=N) + 1.0).astype(np.float32)
    a = (1.0 / np.sqrt(deg)).astype(np.float32)

    shard_of = np.searchsorted(cfg.offs[1:], np.arange(N), side="right")
    w_in = np.arange(N) - cfg.offs[shard_of]
    # partition-major table layout: node (block b, lane p) -> row p*NBLK + b
    trow = shard_of * cfg.SLAB + (w_in % P) * cfg.NBLK + (w_in // P)

    e_core = shard_of[dst]
    e_half = (trow[src] >= cfg.THALF).astype(np.int64)
    e_tix = (trow[src] - e_half * cfg.THALF).astype(np.int64)
    e_blk = ((dst - cfg.offs[e_core]) // P).astype(np.int64)
    e_seg = ((dst - cfg.offs[e_core]) % P).astype(np.int64)

    # sort by src table row within (core, half, blk) for HBM locality
    order = np.lexsort((e_tix, e_blk, e_half, e_core))
    src_s = src[order]
    core_s, half_s, tix_s, blk_s, seg_s = (
        arr[order] for arr in (e_core, e_half, e_tix, e_blk, e_seg))

    counts = np.zeros((cfg.NCORES, 2, cfg.NBLK), np.int64)
    np.add.at(counts, (core_s, half_s, blk_s), 1)
    chunks = -(-counts.max(axis=0) // P)
    cA, cB = chunks[0], chunks[1]

    groups = []
    for g0 in range(0, cfg.NBLK, cfg.GROUP_BLOCKS):
        groups.append(list(range(g0, min(g0 + cfg.GROUP_BLOCKS, cfg.NBLK))))

    TA = int(cA.sum()) * P
    TB = int(cB.sum()) * P
    run_off = np.zeros((cfg.NCORES, 2, cfg.NBLK), np.int64)
    run_off.reshape(-1)[1:] = np.cumsum(counts.reshape(-1))[:-1]

    # LSTM weights with gates reordered (i, f, o, g)
    Wih = np.asarray(W_ih, np.float32).reshape(4, P, P)[GATE_PERM]
    Whh = np.asarray(W_hh, np.float32).reshape(4, P, P)[GATE_PERM]
    bg4 = (np.asarray(b_ih, np.float32)
           + np.asarray(b_hh, np.float32)).reshape(4, P)[GATE_PERM]
    WihT = np.concatenate([w.T for w in Wih], axis=1)  # [P, 4P]
    WhhT = np.concatenate([w.T for w in Whh], axis=1)

    sched_flags = dict(
        has_b=any(np.any(np.asarray(b)) for b in bs),
        has_bg=bool(np.any(np.asarray(b_ih)) or np.any(np.asarray(b_hh))),
    )
    in_maps = []
    for c in range(cfg.NCORES):
        idx_flat = {0: np.zeros(TA, np.int64), 1: np.zeros(TB, np.int64)}
        seg_flat = {0: np.full(TA, PAD_SEG, np.float32),
                    1: np.full(TB, PAD_SEG, np.float32)}
        for h_i, tot_c in enumerate((cA, cB)):
            pos = 0
            for b in range(cfg.NBLK):
                n = int(counts[c, h_i, b])
                o = int(run_off[c, h_i, b])
                idx_flat[h_i][pos:pos + n] = tix_s[o:o + n]
                seg_flat[h_i][pos:pos + n] = seg_s[o:o + n]
                pos += int(tot_c[b]) * P
        idxA = np.zeros((P, max(TA // 16, 1)), np.int16)
        idxB = np.zeros((P, max(TB // 16, 1)), np.int16)
        if TA:
            idxA[:] = np.tile(idx_flat[0].reshape(-1, 16).T.astype(np.int16),
                              (8, 1))
        if TB:
            idxB[:] = np.tile(idx_flat[1].reshape(-1, 16).T.astype(np.int16),
                              (8, 1))
        segA = np.zeros((P, max(TA // P, 1)), ml_dtypes.bfloat16)
        segB = np.zeros((P, max(TB // P, 1)), ml_dtypes.bfloat16)
        if TA:
            segA[:] = seg_flat[0].reshape(-1, P).T.astype(ml_dtypes.bfloat16)
        if TB:
            segB[:] = seg_flat[1].reshape(-1, P).T.astype(ml_dtypes.bfloat16)

        o, s = int(cfg.offs[c]), int(cfg.sizes[c])
        a_pad = np.ones(cfg.SLAB, np.float32)
        a_pad[:s] = a[o:o + s]
        xTb = np.zeros((P, cfg.SLAB), ml_dtypes.bfloat16)
        xTb[:, :s] = np.asarray(x[o:o + s], np.float32).T.astype(
            ml_dtypes.bfloat16)
        a_b = np.tile(a_pad, (P, 1)).astype(ml_dtypes.bfloat16)
        aT = np.ascontiguousarray(
            a_pad.reshape(cfg.NBLK, P).T)          # [P, NBLK]
        a2T = np.ascontiguousarray((a_pad * a_pad).reshape(cfg.NBLK, P).T)
        invaT = (1.0 / a_pad).reshape(1, cfg.SLAB).astype(ml_dtypes.bfloat16)

        m = {
            "xTb": xTb, "a_b": a_b, "aT": aT, "a2T": a2T,
            "idxA": idxA, "idxB": idxB, "segA": segA, "segB": segB,
            "iota": np.tile(np.arange(P, dtype=np.float32), (P, 1)).astype(
                ml_dtypes.bfloat16),
            "ident": np.eye(P, dtype=ml_dtypes.bfloat16),
            "WihT": WihT.astype(ml_dtypes.bfloat16),
            "WhhT": WhhT.astype(ml_dtypes.bfloat16),
            "linW": np.asarray(lin_W, np.float32).astype(ml_dtypes.bfloat16),
            "linb": np.asarray(lin_b, np.float32).reshape(1, cfg.C),
            "ones1": np.ones((1, P), np.float32),
        }
        if sched_flags["has_b"]:
            m["invaT"] = invaT
        if sched_flags["has_bg"]:
            m["bgrow"] = np.ascontiguousarray(
                bg4.reshape(1, 4 * P)).astype(np.float32)
        for li in range(4):
            m[f"W{li}"] = np.asarray(Ws[li], np.float32).astype(
                ml_dtypes.bfloat16)
            if sched_flags["has_b"]:
                m[f"brow{li}"] = np.asarray(bs[li], np.float32).reshape(
                    1, P).astype(ml_dtypes.bfloat16)
        in_maps.append(m)

    sched = dict(cA=cA.astype(np.int64), cB=cB.astype(np.int64),
                 groups=groups, TA=TA, TB=TB, **sched_flags)
    return in_maps, sched


def build_program(cfg, sched, trn_type="TRN2", debug=False):
    single_packet = os.environ.get("GNN_SP", "0") == "1"
    nc = bacc.Bacc(trn_type, target_bir_lowering=False, debug=debug,
                   num_devices=cfg.NCORES, num_swdge_queues=4)
    cA, cB, groups = sched["cA"], sched["cB"], sched["groups"]
    TA, TB = sched["TA"], sched["TB"]
    SLAB, NBLK, THALF, NG = cfg.SLAB, cfg.NBLK, cfg.THALF, cfg.NG

    def din(name, shape, dt):
        return nc.dram_tensor(name, shape, dt, kind="ExternalInput")

    has_b = sched["has_b"]
    has_bg = sched["has_bg"]
    xTb_d = din("xTb", [P, SLAB], BF16)
    a_d = din("a_b", [P, SLAB], BF16)
    aT_d = din("aT", [P, NBLK], F32)
    a2T_d = din("a2T", [P, NBLK], F32)
    invaT_d = din("invaT", [1, SLAB], BF16) if has_b else None
    idxA_d = din("idxA", [P, max(TA // 16, 1)], I16)
    idxB_d = din("idxB", [P, max(TB // 16, 1)], I16)
    segA_d = din("segA", [P, max(TA // P, 1)], BF16)
    segB_d = din("segB", [P, max(TB // P, 1)], BF16)
    iota_d = din("iota", [P, P], BF16)
    ident_d = din("ident", [P, P], BF16)
    W_d = [din(f"W{li}", [P, P], BF16) for li in range(4)]
    brow_d = ([din(f"brow{li}", [1, P], BF16) for li in range(4)]
              if has_b else None)
    WihT_d = din("WihT", [P, 4 * P], BF16)
    WhhT_d = din("WhhT", [P, 4 * P], BF16)
    bgrow_d = din("bgrow", [1, 4 * P], F32) if has_bg else None
    linW_d = din("linW", [P, cfg.C], BF16)
    linb_d = din("linb", [1, cfg.C], F32)
    ones1_d = din("ones1", [1, P], F32)
    out_d = nc.dram_tensor("out", [NG, cfg.C], F32, kind="ExternalOutput")

    rg = [list(range(cfg.NCORES))]

    with tile.TileContext(nc) as tc:
        with tc.tile_pool(name="dram", bufs=1, space="DRAM") as dpool, \
             tc.tile_pool(name="const", bufs=1) as cpool, \
             tc.tile_pool(name="state", bufs=1) as spool, \
             tc.tile_pool(name="gat", bufs=int(os.environ.get("GNN_BUFS", "3"))) as gpool, \
             tc.tile_pool(name="work", bufs=2) as wpool, \
             tc.tile_pool(name="pagg", bufs=4, space="PSUM") as ppool, \
             tc.tile_pool(name="pu", bufs=2, space="PSUM") as upool, \
             tc.tile_pool(name="plstm", bufs=2, space="PSUM") as lpool:

            def cload(dram, shape, dt, tag):
                t = cpool.tile(shape, dt, tag=tag)
                nc.sync.dma_start(t[:], dram[:])
                return t
            iota_t = cload(iota_d, [P, P], BF16, "c_iota")
            ident_t = cload(ident_d, [P, P], BF16, "c_ident")
            W_t = [cload(W_d[i], [P, P], BF16, f"c_W{i}") for i in range(4)]
            brow_t = ([cload(brow_d[i], [1, P], BF16, f"c_b{i}")
                       for i in range(4)] if has_b else None)
            WihT_t = cload(WihT_d, [P, 4 * P], BF16, "c_wih")
            WhhT_t = cload(WhhT_d, [P, 4 * P], BF16, "c_whh")
            bgrow_t = (cload(bgrow_d, [1, 4 * P], F32, "c_bgrow")
                       if has_bg else None)
            linW_t = cload(linW_d, [P, cfg.C], BF16, "c_linw")
            linb_t = cload(linb_d, [1, cfg.C], F32, "c_linb")
            ones1_t = cload(ones1_d, [1, P], F32, "c_ones1")
            aT_t = cload(aT_d, [P, NBLK], F32, "c_aT")
            a2T_t = cload(a2T_d, [P, NBLK], F32, "c_a2T")
            invaT_t = (cload(invaT_d, [1, SLAB], BF16, "c_invaT")
                       if has_b else None)
            a_t = cload(a_d, [P, SLAB], BF16, "c_a")

            slab_sb = [spool.tile([P, SLAB], BF16, tag=f"slab{i}",
                                  name=f"slab{i}")
                       for i in range(2)]
            gbuf = spool.tile([P, SLAB], BF16, tag="gbuf")
            h4a = slab_sb[0]  # free after layer 3; reused for LSTM input

            # ---- layer-0 projection: slab0 = a * (x @ W1), [node, feat] ----
            for j in range(0, SLAB, 512):
                w = min(512, SLAB - j)
                xc = wpool.tile([P, 512], BF16, tag="xchunk")
                nc.sync.dma_start(xc[:, :w], xTb_d[:, j:j + w])
                for i in range(w // P):
                    k = j // P + i
                    pu = upool.tile([P, P], F32, tag="u", space="PSUM")
                    nc.tensor.matmul(pu[:], lhsT=xc[:, i * P:(i + 1) * P],
                                     rhs=W_t[0][:], start=True, stop=True)
                    nc.scalar.activation(
                        out=slab_sb[0][:, k * P:(k + 1) * P], in_=pu[:],
                        func=AF.Identity, scale=aT_t[:, k:k + 1])

            def push_table(li):
                par = li % 2
                slab_dram = dpool.tile([SLAB, P], BF16, tag="slab_dram",
                                       bufs=2)
                table_dram = dpool.tile([cfg.NCORES * SLAB, P], BF16,
                                        addr_space="Shared", tag="table",
                                        bufs=2)
                nc.sync.dma_start(
                    slab_dram[:].rearrange("(p b) f -> p b f", b=NBLK),
                    slab_sb[par][:].rearrange("p (b f) -> p b f", f=P))
                nc.gpsimd.collective_compute(
                    "AllGather", mybir.AluOpType.bypass,
                    replica_groups=rg,
                    ins=[slab_dram[:]],
                    outs=[table_dram[:]],
                )
                return table_dram

            table_t = push_table(0)

            gq = 0
            for li in range(4):
                cur = slab_sb[li % 2]
                nxt = slab_sb[(li + 1) % 2]
                ao = 0
                bo = 0
                pending_u = []  # blocks whose next-layer projection is due

                def do_u(b):
                    if li < 3:
                        pu = upool.tile([P, P], F32, tag="u", space="PSUM")
                        nc.tensor.matmul(pu[:],
                                         lhsT=gbuf[:, b * P:(b + 1) * P],
                                         rhs=W_t[li + 1][:],
                                         start=True, stop=True)
                        nc.scalar.activation(
                            out=nxt[:, b * P:(b + 1) * P], in_=pu[:],
                            func=AF.Identity, scale=a2T_t[:, b:b + 1])
                    else:
                        nc.vector.tensor_tensor(
                            out=h4a[:, b * P:(b + 1) * P],
                            in0=gbuf[:, b * P:(b + 1) * P],
                            in1=a_t[:, b * P:(b + 1) * P],
                            op=OP.mult)

                for blks in groups:
                    nca = int(cA[blks].sum())
                    ncb = int(cB[blks].sum())
                    gx = {}
                    stg = {}
                    for half, ncnt, idxd, segd, off in (
                            (0, nca, idxA_d, segA_d, ao),
                            (1, ncb, idxB_d, segB_d, bo)):
                        if ncnt == 0:
                            continue
                        it = gpool.tile([P, ncnt * 8], I16, tag=f"idx{half}")
                        nc.sync.dma_start(
                            it[:], idxd[:, off * 8:(off + ncnt) * 8])
                        g = gpool.tile([P, ncnt, P], BF16, tag=f"gx{half}")
                        nc.gpsimd.dma_gather(
                            out_ap=g[:],
                            in_ap=table_t[half * THALF:(half + 1) * THALF, :],
                            idxs_ap=it[:],
                            num_idxs=ncnt * P,
                            num_idxs_reg=ncnt * P,
                            elem_size=P,
                            single_packet=single_packet,
                            queue_num=gq % 4,
                        )
                        gq += 1
                        gx[half] = g
                        st_ = gpool.tile([P, ncnt], BF16, tag=f"seg{half}")
                        nc.sync.dma_start(st_[:], segd[:, off:off + ncnt])
                        sg = gpool.tile([P, ncnt, P], BF16, tag=f"stg{half}")
                        nc.vector.tensor_tensor(
                            out=sg[:],
                            in0=st_[:].rearrange("p (c o) -> p c o", o=1)
                                .to_broadcast((P, ncnt, P)),
                            in1=iota_t[:].rearrange("p (o f) -> p o f", o=1)
                                .to_broadcast((P, ncnt, P)),
                            op=OP.is_equal)
                        stg[half] = sg

                    ca_in_grp = 0
                    cb_in_grp = 0
                    for b in blks:
                        pb = ppool.tile([P, P], F32, tag="agg", space="PSUM")
                        na, nb_ = int(cA[b]), int(cB[b])
                        # self-loop + rank-1 bias b[f]*inva[d]
                        nc.tensor.matmul(pb[:],
                                         lhsT=cur[:, b * P:(b + 1) * P],
                                         rhs=ident_t[:], start=True,
                                         stop=(not has_b and na + nb_ == 0))
                        if has_b:
                            nc.tensor.matmul(
                                pb[:], lhsT=brow_t[li][:],
                                rhs=invaT_t[:, b * P:(b + 1) * P],
                                start=False, stop=(na + nb_ == 0))
                        done = 0
                        for half, cnt, base in ((0, na, ca_in_grp),
                                                (1, nb_, cb_in_grp)):
                            for ci in range(cnt):
                                col = base + ci
                                done += 1
                                nc.tensor.matmul(
                                    pb[:], lhsT=gx[half][:, col, :],
                                    rhs=stg[half][:, col, :],
                                    start=False, stop=(done == na + nb_))
                        ca_in_grp += na
                        cb_in_grp += nb_
                        nc.scalar.activation(
                            out=gbuf[:, b * P:(b + 1) * P], in_=pb[:],
                            func=AF.Relu)
                        pending_u.append(b)
                        if len(pending_u) > 1:
                            do_u(pending_u.pop(0))
                    ao += nca
                    bo += ncb
                for b in pending_u:
                    do_u(b)
                if li < 3:
                    table_t = push_table(li + 1)

            c_t = spool.tile([P, NG], F32, tag="c")
            h_t = spool.tile([P, NG], BF16, tag="h")
            nc.vector.memset(c_t[:], 0.0)
            nc.vector.memset(h_t[:], 0.0)

            for t in range(cfg.L):
                pg = lpool.tile([P, 4 * NG], F32, tag="lstm", space="PSUM")
                xt = h4a[:, t:cfg.S_PAD:cfg.L]
                for q in range(4):
                    nc.tensor.matmul(
                        pg[:, q * NG:(q + 1) * NG],
                        lhsT=WhhT_t[:, q * P:(q + 1) * P],
                        rhs=h_t[:], start=True, stop=False)
                    if has_bg:
                        nc.tensor.matmul(
                            pg[:, q * NG:(q + 1) * NG],
                            lhsT=bgrow_t[:, q * P:(q + 1) * P],
                            rhs=ones1_t[:, :NG],
                            start=False, stop=False)
                    nc.tensor.matmul(
                        pg[:, q * NG:(q + 1) * NG],
                        lhsT=WihT_t[:, q * P:(q + 1) * P],
                        rhs=xt, start=False, stop=True)
                af = wpool.tile([P, 3 * NG], F32, tag="af")
                nc.scalar.activation(out=af[:], in_=pg[:, :3 * NG],
                                     func=AF.Sigmoid)
                gv = wpool.tile([P, NG], F32, tag="gv")
                nc.scalar.activation(out=gv[:], in_=pg[:, 3 * NG:4 * NG],
                                     func=AF.Tanh)
                ig = wpool.tile([P, NG], F32, tag="ig")
                nc.vector.tensor_tensor(out=ig[:], in0=af[:, :NG], in1=gv[:],
                                        op=OP.mult)
                fc = wpool.tile([P, NG], F32, tag="fc")
                nc.vector.tensor_tensor(out=fc[:], in0=af[:, NG:2 * NG],
                                        in1=c_t[:], op=OP.mult)
                nc.vector.tensor_tensor(out=c_t[:], in0=fc[:], in1=ig[:],
                                        op=OP.add)
                tc_ = wpool.tile([P, NG], F32, tag="tc")
                nc.scalar.activation(out=tc_[:], in_=c_t[:], func=AF.Tanh)
                nc.vector.tensor_tensor(out=h_t[:], in0=af[:, 2 * NG:3 * NG],
                                        in1=tc_[:], op=OP.mult)

            po = lpool.tile([P, cfg.C], F32, tag="lstm", space="PSUM")
            nc.tensor.matmul(po[:NG, :], lhsT=h_t[:, :NG], rhs=linW_t[:],
                             start=True, stop=False)
            nc.tensor.matmul(po[:NG, :], lhsT=ones1_t[:, :NG], rhs=linb_t[:],
                             start=False, stop=True)
            os_ = wpool.tile([P, cfg.C], F32, tag="outs")
            nc.scalar.activation(out=os_[:NG, :], in_=po[:NG, :], func=AF.Copy)
            nc.sync.dma_start(out_d[:], os_[:NG, :])

    nc.compile()
    return nc


def assemble(cfg, results):
    out = np.zeros((cfg.G, cfg.C), np.float32)
    for c in range(cfg.NCORES):
        g0 = int(cfg.offs[c]) // cfg.L
        ng = cfg.sizes[c] // cfg.L
        out[g0:g0 + ng] = results[c]["out"][:ng]
    return out


_BUILD_CACHE = {}


def kernel(x, edge_index, batch, W1, b1, W2, b2, W3, b3, W4, b4,
           W_ih, W_hh, b_ih, b_hh, lin_W, lin_b):
    global LAST_RESULTS
    cfg = Config()
    x = np.asarray(x, np.float32)
    edge_index = np.asarray(edge_index, np.int64)
    Ws = [np.asarray(w, np.float32) for w in (W1, W2, W3, W4)]
    bs = [np.asarray(b, np.float32) for b in (b1, b2, b3, b4)]

    in_maps, sched = preprocess(
        cfg, x, edge_index, Ws, bs,
        np.asarray(W_ih, np.float32), np.asarray(W_hh, np.float32),
        np.asarray(b_ih, np.float32), np.asarray(b_hh, np.float32),
        np.asarray(lin_W, np.float32), np.asarray(lin_b, np.float32))

    key = (sched["TA"], sched["TB"], tuple(sched["cA"]), tuple(sched["cB"]),
           sched["has_b"], sched["has_bg"])
    if key not in _BUILD_CACHE:
        _BUILD_CACHE[key] = build_program(cfg, sched)
    nc = _BUILD_CACHE[key]

    res = run_bass_kernel_spmd(nc, in_maps, core_ids=list(range(cfg.NCORES)),
                               trace=TRACE,
                               tmpdir=os.environ.get("GNN_TMPDIR") or None)
    LAST_RESULTS = res
    return assemble(cfg, res.results)
